# revision 5
# baseline (speedup 1.0000x reference)
"""MoE all-reduce + RMSNorm fused kernel for Trainium2 (8 NeuronCores).

Computes, for E=8, T=8192, H=4096 (all fp32):
    expert_reduction = einsum("eth,et->th", active_experts_token_input, scale_input)
    output_residual  = expert_reduction + token_input + residual
    hidden_states    = output_residual * rsqrt(mean(output_residual^2, -1) + 1e-5) * norm_weight
returns (hidden_states, output_residual).

Sharding: tokens (T) split evenly across the 8 cores (data/sequence parallel);
the norm is over H so every core is fully independent — no collectives.

Per-core device program: 8 chunks of 128 tokens (tokens on partitions, H on the
free axis). Per chunk the expert reduction runs as 8 fused DVE
scalar_tensor_tensor MACs (acc = a_e * s_e + acc), the mean-square runs on the
otherwise-idle ACT engine (Square activation with accum_out, scratch in PSUM),
and rsqrt = ACT Sqrt + DVE reciprocal + one Newton step (ACT Sqrt alone is
low-precision). The kernel is HBM-bandwidth-bound: ~192 MiB of DMA per core.
"""

import sys
import numpy as np

try:
    import concourse  # noqa: F401
except ImportError:
    sys.path.insert(0, "/opt/trn_rl_repo")

E, T, H = 8, 8192, 4096
N_CORES = 8
T_CORE = T // N_CORES  # 1024 tokens per core
P = 128                # SBUF partitions = tokens per chunk
N_CHUNKS = T_CORE // P  # 8
EPS = 1e-5

_CACHE = {}


def _build_program():
    from contextlib import ExitStack

    import concourse.bass as bass  # noqa: F401
    from concourse import bacc, mybir, tile

    f32 = mybir.dt.float32
    mult = mybir.AluOpType.mult
    add = mybir.AluOpType.add
    Square = mybir.ActivationFunctionType.Square
    Sqrt = mybir.ActivationFunctionType.Sqrt

    nc = bacc.Bacc(
        "TRN2",
        target_bir_lowering=False,
        debug=False,
        enable_asserts=False,
        num_devices=N_CORES,
    )

    a = nc.dram_tensor("a_in", [E, T_CORE, H], f32, kind="ExternalInput").ap()
    tok = nc.dram_tensor("tok_in", [T_CORE, H], f32, kind="ExternalInput").ap()
    res = nc.dram_tensor("res_in", [T_CORE, H], f32, kind="ExternalInput").ap()
    sc = nc.dram_tensor("sc_in", [T_CORE, E], f32, kind="ExternalInput").ap()
    nw = nc.dram_tensor("nw_in", [P, H], f32, kind="ExternalInput").ap()
    hid_out = nc.dram_tensor("hid_out", [T_CORE, H], f32, kind="ExternalOutput").ap()
    ores_out = nc.dram_tensor("ores_out", [T_CORE, H], f32, kind="ExternalOutput").ap()

    with tile.TileContext(nc) as tc, ExitStack() as ctx:
        nw_pool = ctx.enter_context(tc.tile_pool(name="nw", bufs=1))
        a_pool = ctx.enter_context(tc.tile_pool(name="a", bufs=3))
        tok_pool = ctx.enter_context(tc.tile_pool(name="tok", bufs=2))
        res_pool = ctx.enter_context(tc.tile_pool(name="res", bufs=2))
        acc_pool = ctx.enter_context(tc.tile_pool(name="acc", bufs=2))
        hid_pool = ctx.enter_context(tc.tile_pool(name="hid", bufs=1))
        sc_pool = ctx.enter_context(tc.tile_pool(name="sc", bufs=2))
        st_pool = ctx.enter_context(tc.tile_pool(name="st", bufs=2))
        ps_pool = ctx.enter_context(tc.tile_pool(name="ps", bufs=1, space="PSUM"))

        nw_t = nw_pool.tile([P, H], f32)
        nc.sync.dma_start(out=nw_t[:], in_=nw[:, :])

        # const per-partition scalars for ACT bias operands (no const-AP db here)
        zero_t = nw_pool.tile([P, 1], f32, tag="zero")
        nc.vector.memset(zero_t[:], 0.0)
        eps_t = nw_pool.tile([P, 1], f32, tag="eps")
        nc.vector.memset(eps_t[:], EPS)

        for c in range(N_CHUNKS):
            t0 = c * P
            sc_t = sc_pool.tile([P, E], f32)
            nc.sync.dma_start(out=sc_t[:], in_=sc[t0 : t0 + P, :])
            tok_t = tok_pool.tile([P, H], f32)
            nc.sync.dma_start(out=tok_t[:], in_=tok[t0 : t0 + P, :])
            res_t = res_pool.tile([P, H], f32)
            nc.sync.dma_start(out=res_t[:], in_=res[t0 : t0 + P, :])

            acc_t = acc_pool.tile([P, H], f32)
            for e in range(E):
                a_t = a_pool.tile([P, H], f32, tag="a_t")
                nc.sync.dma_start(out=a_t[:], in_=a[e, t0 : t0 + P, :])
                prev = tok_t if e == 0 else acc_t
                nc.vector.scalar_tensor_tensor(
                    out=acc_t[:],
                    in0=a_t[:],
                    scalar=sc_t[:, e : e + 1],
                    in1=prev[:],
                    op0=mult,
                    op1=add,
                )
            nc.vector.tensor_tensor(out=acc_t[:], in0=acc_t[:], in1=res_t[:], op=add)
            nc.sync.dma_start(out=ores_out[t0 : t0 + P, :], in_=acc_t[:])

            # mean(acc^2) on ACT: Square(acc/64) summed over H -> sum(acc^2)/4096
            var_t = st_pool.tile([P, 1], f32)
            sq_t = ps_pool.tile([P, H], f32)
            nc.scalar.activation(
                out=sq_t[:], in_=acc_t[:], func=Square, scale=1.0 / 64.0,
                bias=zero_t[:, 0:1], accum_out=var_t[:],
            )
            # rsqrt(var + eps): ACT Sqrt seed + DVE reciprocal + 1 Newton step
            std_t = st_pool.tile([P, 1], f32)
            nc.scalar.activation(
                out=std_t[:], in_=var_t[:], func=Sqrt, bias=eps_t[:, 0:1]
            )
            y_t = st_pool.tile([P, 1], f32)
            nc.vector.reciprocal(out=y_t[:], in_=std_t[:])
            x_t = st_pool.tile([P, 1], f32)
            nc.vector.tensor_scalar_add(x_t[:], var_t[:], EPS)
            t_t = st_pool.tile([P, 1], f32)
            nc.vector.tensor_tensor(out=t_t[:], in0=y_t[:], in1=y_t[:], op=mult)
            nc.vector.tensor_tensor(out=t_t[:], in0=t_t[:], in1=x_t[:], op=mult)
            h_t = st_pool.tile([P, 1], f32)
            nc.vector.tensor_scalar(
                out=h_t[:], in0=t_t[:], scalar1=-0.5, scalar2=1.5, op0=mult, op1=add
            )
            y2_t = st_pool.tile([P, 1], f32)
            nc.vector.tensor_tensor(out=y2_t[:], in0=y_t[:], in1=h_t[:], op=mult)

            hid_t = hid_pool.tile([P, H], f32)
            nc.vector.scalar_tensor_tensor(
                out=hid_t[:],
                in0=acc_t[:],
                scalar=y2_t[:, 0:1],
                in1=nw_t[:],
                op0=mult,
                op1=mult,
            )
            nc.sync.dma_start(out=hid_out[t0 : t0 + P, :], in_=hid_t[:])

    nc.compile()
    return nc


def _get_program():
    if "nc" not in _CACHE:
        _CACHE["nc"] = _build_program()
    return _CACHE["nc"]


def _make_in_maps(residual, norm_weight, scale_input, active, token_input):
    nw_b = np.ascontiguousarray(
        np.broadcast_to(np.asarray(norm_weight, np.float32), (P, H))
    )
    in_maps = []
    for c in range(N_CORES):
        lo, hi = c * T_CORE, (c + 1) * T_CORE
        in_maps.append(
            {
                "a_in": np.ascontiguousarray(active[:, lo:hi, :], np.float32),
                "tok_in": np.ascontiguousarray(token_input[lo:hi], np.float32),
                "res_in": np.ascontiguousarray(residual[lo:hi], np.float32),
                "sc_in": np.ascontiguousarray(scale_input[:, lo:hi].T, np.float32),
                "nw_in": nw_b,
            }
        )
    return in_maps


def _ensure_ntff_hook():
    """Register the axon NTFF profiling hook if the image's antenv lacks it."""
    import types

    name = "antenv.axon_hooks"
    if name in sys.modules:
        return
    try:
        import antenv.axon_hooks  # noqa: F401

        return
    except ImportError:
        pass
    mod = types.ModuleType(name)
    mod._hook = None
    mod.set_axon_ntff_profile_hook = lambda h: setattr(mod, "_hook", h)
    mod.get_axon_ntff_profile_hook = lambda: mod._hook
    sys.modules[name] = mod
    try:
        from trn_agent_boot.trn_boot import _ntff_profile_via_ctypes

        h = _ntff_profile_via_ctypes("/opt/axon/libaxon_pjrt.so")
        if h is not None:
            mod._hook = h
    except Exception:
        pass


def kernel(
    residual,
    norm_weight,
    scale_input,
    active_experts_token_input,
    token_input,
    device_num_experts,
    _trace=False,
):
    if _trace:
        _ensure_ntff_hook()
    from concourse.bass_utils import run_bass_kernel_spmd

    assert int(device_num_experts) == E
    residual = np.asarray(residual, np.float32)
    norm_weight = np.asarray(norm_weight, np.float32)
    scale_input = np.asarray(scale_input, np.float32)
    active = np.asarray(active_experts_token_input, np.float32)
    token_input = np.asarray(token_input, np.float32)

    nc = _get_program()
    in_maps = _make_in_maps(residual, norm_weight, scale_input, active, token_input)
    r = run_bass_kernel_spmd(nc, in_maps, list(range(N_CORES)), trace=_trace)
    hidden = np.concatenate([r.results[c]["hid_out"] for c in range(N_CORES)], axis=0)
    outres = np.concatenate([r.results[c]["ores_out"] for c in range(N_CORES)], axis=0)
    if _trace:
        _CACHE["last_result"] = r
    return hidden, outres


# revision 10
# speedup vs baseline: 1.0475x; 1.0475x over previous
"""MoE all-reduce + RMSNorm fused kernel for Trainium2 (8 NeuronCores).

Computes, for E=8, T=8192, H=4096 (all fp32):
    expert_reduction = einsum("eth,et->th", active_experts_token_input, scale_input)
    output_residual  = expert_reduction + token_input + residual
    hidden_states    = output_residual * rsqrt(mean(output_residual^2, -1) + 1e-5) * norm_weight
returns (hidden_states, output_residual).

Sharding: tokens (T) split evenly across the 8 cores (data/sequence parallel);
the norm is over H so every core is fully independent — no collectives.

Per-core device program: 8 chunks of 128 tokens (tokens on partitions, H on the
free axis). Per chunk the expert reduction runs as 8 fused DVE
scalar_tensor_tensor MACs (acc = a_e * s_e + acc), the mean-square runs on the
otherwise-idle ACT engine (Square activation with accum_out, scratch in PSUM),
and rsqrt = ACT Sqrt + DVE reciprocal + one Newton step (ACT Sqrt alone is
low-precision). The kernel is HBM-bandwidth-bound: ~192 MiB of DMA per core.
"""

import sys
import numpy as np

try:
    import concourse  # noqa: F401
except ImportError:
    sys.path.insert(0, "/opt/trn_rl_repo")

E, T, H = 8, 8192, 4096
N_CORES = 8
T_CORE = T // N_CORES  # 1024 tokens per core
P = 128                # SBUF partitions = tokens per chunk
N_CHUNKS = T_CORE // P  # 8
EPS = 1e-5

_CACHE = {}


def _build_program():
    from contextlib import ExitStack

    import concourse.bass as bass  # noqa: F401
    from concourse import bacc, mybir, tile

    f32 = mybir.dt.float32
    mult = mybir.AluOpType.mult
    add = mybir.AluOpType.add
    Square = mybir.ActivationFunctionType.Square
    Sqrt = mybir.ActivationFunctionType.Sqrt

    nc = bacc.Bacc(
        "TRN2",
        target_bir_lowering=False,
        debug=False,
        enable_asserts=False,
        num_devices=N_CORES,
    )

    a = nc.dram_tensor("a_in", [E, T_CORE, H], f32, kind="ExternalInput").ap()
    tok = nc.dram_tensor("tok_in", [T_CORE, H], f32, kind="ExternalInput").ap()
    res = nc.dram_tensor("res_in", [T_CORE, H], f32, kind="ExternalInput").ap()
    # scales pre-packed host-side as [P, N_CHUNKS*E]: col c*E+e = scale[e, c*128+p]
    sc = nc.dram_tensor("sc_in", [P, N_CHUNKS * E], f32, kind="ExternalInput").ap()
    nw = nc.dram_tensor("nw_in", [P, H], f32, kind="ExternalInput").ap()
    hid_out = nc.dram_tensor("hid_out", [T_CORE, H], f32, kind="ExternalOutput").ap()
    ores_out = nc.dram_tensor("ores_out", [T_CORE, H], f32, kind="ExternalOutput").ap()

    with tile.TileContext(nc) as tc, ExitStack() as ctx:
        nw_pool = ctx.enter_context(tc.tile_pool(name="nw", bufs=1))
        a_pool = ctx.enter_context(tc.tile_pool(name="a", bufs=4))
        tr_pool = ctx.enter_context(tc.tile_pool(name="tr", bufs=3))
        acc_pool = ctx.enter_context(tc.tile_pool(name="acc", bufs=2))
        hid_pool = ctx.enter_context(tc.tile_pool(name="hid", bufs=1))
        st_pool = ctx.enter_context(tc.tile_pool(name="st", bufs=2))
        ps_pool = ctx.enter_context(tc.tile_pool(name="ps", bufs=1, space="PSUM"))

        # one-time preloads: norm weight (broadcast) + all per-token scales
        sc_t = nw_pool.tile([P, N_CHUNKS * E], f32, tag="sc")
        nc.sync.dma_start(out=sc_t[:], in_=sc[:, :])
        nw_t = nw_pool.tile([P, H], f32)
        nc.sync.dma_start(out=nw_t[:], in_=nw[:, :])

        # const per-partition scalars for ACT bias operands (no const-AP db here)
        zero_t = nw_pool.tile([P, 1], f32, tag="zero")
        nc.vector.memset(zero_t[:], 0.0)
        eps_t = nw_pool.tile([P, 1], f32, tag="eps")
        nc.vector.memset(eps_t[:], EPS)

        for c in range(N_CHUNKS):
            t0 = c * P
            tok_t = tr_pool.tile([P, H], f32, tag="tr")
            nc.sync.dma_start(out=tok_t[:], in_=tok[t0 : t0 + P, :])
            res_t = tr_pool.tile([P, H], f32, tag="tr")
            nc.sync.dma_start(out=res_t[:], in_=res[t0 : t0 + P, :])

            acc_t = acc_pool.tile([P, H], f32)
            for e in range(E):
                a_t = a_pool.tile([P, H], f32, tag="a_t")
                nc.sync.dma_start(out=a_t[:], in_=a[e, t0 : t0 + P, :])
                prev = tok_t if e == 0 else acc_t
                nc.vector.scalar_tensor_tensor(
                    out=acc_t[:],
                    in0=a_t[:],
                    scalar=sc_t[:, c * E + e : c * E + e + 1],
                    in1=prev[:],
                    op0=mult,
                    op1=add,
                )
            nc.vector.tensor_tensor(out=acc_t[:], in0=acc_t[:], in1=res_t[:], op=add)
            nc.gpsimd.dma_start(out=ores_out[t0 : t0 + P, :], in_=acc_t[:])

            # mean(acc^2) on ACT: Square(acc/64) summed over H -> sum(acc^2)/4096
            var_t = st_pool.tile([P, 1], f32)
            sq_t = ps_pool.tile([P, H], f32)
            nc.scalar.activation(
                out=sq_t[:], in_=acc_t[:], func=Square, scale=1.0 / 64.0,
                bias=zero_t[:, 0:1], accum_out=var_t[:],
            )
            # rsqrt(var + eps): ACT Sqrt seed + DVE reciprocal + 1 Newton step
            std_t = st_pool.tile([P, 1], f32)
            nc.scalar.activation(
                out=std_t[:], in_=var_t[:], func=Sqrt, bias=eps_t[:, 0:1]
            )
            y_t = st_pool.tile([P, 1], f32)
            nc.vector.reciprocal(out=y_t[:], in_=std_t[:])
            x_t = st_pool.tile([P, 1], f32)
            nc.vector.tensor_scalar_add(x_t[:], var_t[:], EPS)
            t_t = st_pool.tile([P, 1], f32)
            nc.vector.tensor_tensor(out=t_t[:], in0=y_t[:], in1=y_t[:], op=mult)
            nc.vector.tensor_tensor(out=t_t[:], in0=t_t[:], in1=x_t[:], op=mult)
            h_t = st_pool.tile([P, 1], f32)
            nc.vector.tensor_scalar(
                out=h_t[:], in0=t_t[:], scalar1=-0.5, scalar2=1.5, op0=mult, op1=add
            )
            y2_t = st_pool.tile([P, 1], f32)
            nc.vector.tensor_tensor(out=y2_t[:], in0=y_t[:], in1=h_t[:], op=mult)

            hid_t = hid_pool.tile([P, H], f32)
            nc.vector.scalar_tensor_tensor(
                out=hid_t[:],
                in0=acc_t[:],
                scalar=y2_t[:, 0:1],
                in1=nw_t[:],
                op0=mult,
                op1=mult,
            )
            nc.gpsimd.dma_start(out=hid_out[t0 : t0 + P, :], in_=hid_t[:])

    nc.compile()
    return nc


def _get_program():
    if "nc" not in _CACHE:
        _CACHE["nc"] = _build_program()
    return _CACHE["nc"]


def _make_in_maps(residual, norm_weight, scale_input, active, token_input):
    nw_b = np.ascontiguousarray(
        np.broadcast_to(np.asarray(norm_weight, np.float32), (P, H))
    )
    in_maps = []
    for c in range(N_CORES):
        lo, hi = c * T_CORE, (c + 1) * T_CORE
        in_maps.append(
            {
                "a_in": np.ascontiguousarray(active[:, lo:hi, :], np.float32),
                "tok_in": np.ascontiguousarray(token_input[lo:hi], np.float32),
                "res_in": np.ascontiguousarray(residual[lo:hi], np.float32),
                "sc_in": np.ascontiguousarray(
                    scale_input[:, lo:hi]
                    .reshape(E, N_CHUNKS, P)
                    .transpose(2, 1, 0)
                    .reshape(P, N_CHUNKS * E),
                    np.float32,
                ),
                "nw_in": nw_b,
            }
        )
    return in_maps


def _ensure_ntff_hook():
    """Register the axon NTFF profiling hook if the image's antenv lacks it."""
    import types

    name = "antenv.axon_hooks"
    if name in sys.modules:
        return
    try:
        import antenv.axon_hooks  # noqa: F401

        return
    except ImportError:
        pass
    mod = types.ModuleType(name)
    mod._hook = None
    mod.set_axon_ntff_profile_hook = lambda h: setattr(mod, "_hook", h)
    mod.get_axon_ntff_profile_hook = lambda: mod._hook
    sys.modules[name] = mod
    try:
        from trn_agent_boot.trn_boot import _ntff_profile_via_ctypes

        h = _ntff_profile_via_ctypes("/opt/axon/libaxon_pjrt.so")
        if h is not None:
            mod._hook = h
    except Exception:
        pass


def kernel(
    residual,
    norm_weight,
    scale_input,
    active_experts_token_input,
    token_input,
    device_num_experts,
    _trace=False,
):
    if _trace:
        _ensure_ntff_hook()
    from concourse.bass_utils import run_bass_kernel_spmd

    assert int(device_num_experts) == E
    residual = np.asarray(residual, np.float32)
    norm_weight = np.asarray(norm_weight, np.float32)
    scale_input = np.asarray(scale_input, np.float32)
    active = np.asarray(active_experts_token_input, np.float32)
    token_input = np.asarray(token_input, np.float32)

    nc = _get_program()
    in_maps = _make_in_maps(residual, norm_weight, scale_input, active, token_input)
    r = run_bass_kernel_spmd(nc, in_maps, list(range(N_CORES)), trace=_trace)
    hidden = np.concatenate([r.results[c]["hid_out"] for c in range(N_CORES)], axis=0)
    outres = np.concatenate([r.results[c]["ores_out"] for c in range(N_CORES)], axis=0)
    if _trace:
        _CACHE["last_result"] = r
    return hidden, outres


# revision 16
# speedup vs baseline: 1.0523x; 1.0047x over previous
"""MoE all-reduce + RMSNorm fused kernel for Trainium2 (8 NeuronCores).

Computes, for E=8, T=8192, H=4096 (all fp32):
    expert_reduction = einsum("eth,et->th", active_experts_token_input, scale_input)
    output_residual  = expert_reduction + token_input + residual
    hidden_states    = output_residual * rsqrt(mean(output_residual^2, -1) + 1e-5) * norm_weight
returns (hidden_states, output_residual).

Sharding: tokens (T) split evenly across the 8 cores (data/sequence parallel);
the norm is over H so every core is fully independent — no collectives.

Per-core device program: 8 chunks of 128 tokens (tokens on partitions, H on the
free axis). Per chunk the expert reduction runs as 8 fused DVE
scalar_tensor_tensor MACs (acc = a_e * s_e + acc), the mean-square runs on the
otherwise-idle ACT engine (Square activation with accum_out, scratch in PSUM),
and rsqrt = ACT Sqrt + DVE reciprocal + one Newton step (ACT Sqrt alone is
low-precision). The kernel is HBM-bandwidth-bound: ~192 MiB of DMA per core.
"""

import sys
import numpy as np

try:
    import concourse  # noqa: F401
except ImportError:
    sys.path.insert(0, "/opt/trn_rl_repo")

E, T, H = 8, 8192, 4096
N_CORES = 8
T_CORE = T // N_CORES  # 1024 tokens per core
P = 128                # SBUF partitions = tokens per chunk
N_CHUNKS = T_CORE // P  # 8
EPS = 1e-5

_CACHE = {}


def _build_program():
    from contextlib import ExitStack

    import concourse.bass as bass  # noqa: F401
    from concourse import bacc, mybir, tile

    f32 = mybir.dt.float32
    mult = mybir.AluOpType.mult
    add = mybir.AluOpType.add
    Square = mybir.ActivationFunctionType.Square
    Sqrt = mybir.ActivationFunctionType.Sqrt

    nc = bacc.Bacc(
        "TRN2",
        target_bir_lowering=False,
        debug=False,
        enable_asserts=False,
        num_devices=N_CORES,
    )

    a = nc.dram_tensor("a_in", [E, T_CORE, H], f32, kind="ExternalInput").ap()
    tok = nc.dram_tensor("tok_in", [T_CORE, H], f32, kind="ExternalInput").ap()
    res = nc.dram_tensor("res_in", [T_CORE, H], f32, kind="ExternalInput").ap()
    # scales pre-packed host-side as [P, N_CHUNKS*E]: col c*E+e = scale[e, c*128+p]
    sc = nc.dram_tensor("sc_in", [P, N_CHUNKS * E], f32, kind="ExternalInput").ap()
    nw = nc.dram_tensor("nw_in", [P, H], f32, kind="ExternalInput").ap()
    hid_out = nc.dram_tensor("hid_out", [T_CORE, H], f32, kind="ExternalOutput").ap()
    ores_out = nc.dram_tensor("ores_out", [T_CORE, H], f32, kind="ExternalOutput").ap()

    with tile.TileContext(nc) as tc, ExitStack() as ctx:
        nw_pool = ctx.enter_context(tc.tile_pool(name="nw", bufs=1))
        a_pool = ctx.enter_context(tc.tile_pool(name="a", bufs=4))
        tr_pool = ctx.enter_context(tc.tile_pool(name="tr", bufs=3))
        acc_pool = ctx.enter_context(tc.tile_pool(name="acc", bufs=2))
        hid_pool = ctx.enter_context(tc.tile_pool(name="hid", bufs=1))
        st_pool = ctx.enter_context(tc.tile_pool(name="st", bufs=2))
        ps_pool = ctx.enter_context(tc.tile_pool(name="ps", bufs=1, space="PSUM"))

        # one-time preloads on the SWDGE path (keep the HWDGE load FIFO clean)
        sc_t = nw_pool.tile([P, N_CHUNKS * E], f32, tag="sc")
        nc.gpsimd.dma_start(out=sc_t[:], in_=sc[:, :])
        nw_t = nw_pool.tile([P, H], f32)
        nc.gpsimd.dma_start(out=nw_t[:], in_=nw[:, :])

        # const per-partition scalars for ACT bias operands (no const-AP db here)
        zero_t = nw_pool.tile([P, 1], f32, tag="zero")
        nc.vector.memset(zero_t[:], 0.0)
        eps_t = nw_pool.tile([P, 1], f32, tag="eps")
        nc.vector.memset(eps_t[:], EPS)

        for c in range(N_CHUNKS):
            t0 = c * P
            tok_t = tr_pool.tile([P, H], f32, tag="tr")
            nc.sync.dma_start(out=tok_t[:], in_=tok[t0 : t0 + P, :])
            res_t = tr_pool.tile([P, H], f32, tag="tr")
            nc.sync.dma_start(out=res_t[:], in_=res[t0 : t0 + P, :])

            # last chunk runs in two H-halves to shorten the kernel tail
            splits = [(0, H)] if c < N_CHUNKS - 1 else [(0, H // 2), (H // 2, H // 2)]
            acc_t = acc_pool.tile([P, H], f32)
            var_parts = []
            for off, w in splits:
                cols = slice(off, off + w)
                for e in range(E):
                    a_t = a_pool.tile([P, H], f32, tag="a_t")
                    nc.sync.dma_start(
                        out=a_t[:, 0:w], in_=a[e, t0 : t0 + P, cols]
                    )
                    prev_ap = tok_t[:, cols] if e == 0 else acc_t[:, cols]
                    nc.vector.scalar_tensor_tensor(
                        out=acc_t[:, cols],
                        in0=a_t[:, 0:w],
                        scalar=sc_t[:, c * E + e : c * E + e + 1],
                        in1=prev_ap,
                        op0=mult,
                        op1=add,
                    )
                # residual add on the otherwise-idle GpSimd engine (frees DVE)
                nc.gpsimd.tensor_tensor(
                    out=acc_t[:, cols], in0=acc_t[:, cols], in1=res_t[:, cols], op=add
                )
                nc.gpsimd.dma_start(out=ores_out[t0 : t0 + P, cols], in_=acc_t[:, cols])

                # partial mean-square on ACT: sum(Square(acc/64)) = sum(acc^2)/4096
                var_t = st_pool.tile([P, 1], f32, tag="var")
                sq_t = ps_pool.tile([P, H], f32, tag="sq")
                nc.scalar.activation(
                    out=sq_t[:, 0:w], in_=acc_t[:, cols], func=Square,
                    scale=1.0 / 64.0, bias=zero_t[:, 0:1], accum_out=var_t[:],
                )
                var_parts.append(var_t)

            if len(var_parts) > 1:
                vsum_t = st_pool.tile([P, 1], f32, tag="vsum")
                nc.vector.tensor_tensor(
                    out=vsum_t[:], in0=var_parts[0][:], in1=var_parts[1][:], op=add
                )
                var_t = vsum_t
            else:
                var_t = var_parts[0]

            # rsqrt(var + eps): ACT Sqrt seed + DVE reciprocal + 1 Newton step
            std_t = st_pool.tile([P, 1], f32)
            nc.scalar.activation(
                out=std_t[:], in_=var_t[:], func=Sqrt, bias=eps_t[:, 0:1]
            )
            y_t = st_pool.tile([P, 1], f32)
            nc.vector.reciprocal(out=y_t[:], in_=std_t[:])
            x_t = st_pool.tile([P, 1], f32)
            nc.vector.tensor_scalar_add(x_t[:], var_t[:], EPS)
            t_t = st_pool.tile([P, 1], f32)
            nc.vector.tensor_tensor(out=t_t[:], in0=y_t[:], in1=y_t[:], op=mult)
            nc.vector.tensor_tensor(out=t_t[:], in0=t_t[:], in1=x_t[:], op=mult)
            h_t = st_pool.tile([P, 1], f32)
            nc.vector.tensor_scalar(
                out=h_t[:], in0=t_t[:], scalar1=-0.5, scalar2=1.5, op0=mult, op1=add
            )
            y2_t = st_pool.tile([P, 1], f32)
            nc.vector.tensor_tensor(out=y2_t[:], in0=y_t[:], in1=h_t[:], op=mult)

            hid_t = hid_pool.tile([P, H], f32)
            for off, w in splits:
                cols = slice(off, off + w)
                nc.vector.scalar_tensor_tensor(
                    out=hid_t[:, cols],
                    in0=acc_t[:, cols],
                    scalar=y2_t[:, 0:1],
                    in1=nw_t[:, cols],
                    op0=mult,
                    op1=mult,
                )
                nc.gpsimd.dma_start(out=hid_out[t0 : t0 + P, cols], in_=hid_t[:, cols])

    nc.compile()
    return nc


def _get_program():
    if "nc" not in _CACHE:
        _CACHE["nc"] = _build_program()
    return _CACHE["nc"]


def _make_in_maps(residual, norm_weight, scale_input, active, token_input):
    nw_b = np.ascontiguousarray(
        np.broadcast_to(np.asarray(norm_weight, np.float32), (P, H))
    )
    in_maps = []
    for c in range(N_CORES):
        lo, hi = c * T_CORE, (c + 1) * T_CORE
        in_maps.append(
            {
                "a_in": np.ascontiguousarray(active[:, lo:hi, :], np.float32),
                "tok_in": np.ascontiguousarray(token_input[lo:hi], np.float32),
                "res_in": np.ascontiguousarray(residual[lo:hi], np.float32),
                "sc_in": np.ascontiguousarray(
                    scale_input[:, lo:hi]
                    .reshape(E, N_CHUNKS, P)
                    .transpose(2, 1, 0)
                    .reshape(P, N_CHUNKS * E),
                    np.float32,
                ),
                "nw_in": nw_b,
            }
        )
    return in_maps


def _ensure_ntff_hook():
    """Register the axon NTFF profiling hook if the image's antenv lacks it."""
    import types

    name = "antenv.axon_hooks"
    if name in sys.modules:
        return
    try:
        import antenv.axon_hooks  # noqa: F401

        return
    except ImportError:
        pass
    mod = types.ModuleType(name)
    mod._hook = None
    mod.set_axon_ntff_profile_hook = lambda h: setattr(mod, "_hook", h)
    mod.get_axon_ntff_profile_hook = lambda: mod._hook
    sys.modules[name] = mod
    try:
        from trn_agent_boot.trn_boot import _ntff_profile_via_ctypes

        h = _ntff_profile_via_ctypes("/opt/axon/libaxon_pjrt.so")
        if h is not None:
            mod._hook = h
    except Exception:
        pass


def kernel(
    residual,
    norm_weight,
    scale_input,
    active_experts_token_input,
    token_input,
    device_num_experts,
    _trace=False,
):
    if _trace:
        _ensure_ntff_hook()
    from concourse.bass_utils import run_bass_kernel_spmd

    assert int(device_num_experts) == E
    residual = np.asarray(residual, np.float32)
    norm_weight = np.asarray(norm_weight, np.float32)
    scale_input = np.asarray(scale_input, np.float32)
    active = np.asarray(active_experts_token_input, np.float32)
    token_input = np.asarray(token_input, np.float32)

    nc = _get_program()
    in_maps = _make_in_maps(residual, norm_weight, scale_input, active, token_input)
    r = run_bass_kernel_spmd(nc, in_maps, list(range(N_CORES)), trace=_trace)
    hidden = np.concatenate([r.results[c]["hid_out"] for c in range(N_CORES)], axis=0)
    outres = np.concatenate([r.results[c]["ores_out"] for c in range(N_CORES)], axis=0)
    if _trace:
        _CACHE["last_result"] = r
    return hidden, outres


# revision 17
# speedup vs baseline: 1.2164x; 1.1559x over previous
"""MoE all-reduce + RMSNorm fused kernel for Trainium2 (8 NeuronCores).

Computes, for E=8, T=8192, H=4096 (all fp32):
    expert_reduction = einsum("eth,et->th", active_experts_token_input, scale_input)
    output_residual  = expert_reduction + token_input + residual
    hidden_states    = output_residual * rsqrt(mean(output_residual^2, -1) + 1e-5) * norm_weight
returns (hidden_states, output_residual).

Sharding: tokens (T) split evenly across the 8 cores (data/sequence parallel);
the norm is over H so every core is fully independent — no collectives.

Per-core device program: 8 chunks of 128 tokens (tokens on partitions, H on the
free axis). Per chunk the expert reduction runs as 8 fused DVE
scalar_tensor_tensor MACs (acc = a_e * s_e + acc), the mean-square runs on the
otherwise-idle ACT engine (Square activation with accum_out, scratch in PSUM),
and rsqrt = ACT Sqrt + DVE reciprocal + one Newton step (ACT Sqrt alone is
low-precision). The kernel is HBM-bandwidth-bound: ~192 MiB of DMA per core.
"""

import sys
import numpy as np

try:
    import concourse  # noqa: F401
except ImportError:
    sys.path.insert(0, "/opt/trn_rl_repo")

E, T, H = 8, 8192, 4096
N_CORES = 8
T_CORE = T // N_CORES  # 1024 tokens per core
P = 128                # SBUF partitions = tokens per chunk
N_CHUNKS = T_CORE // P  # 8
EPS = 1e-5

_CACHE = {}


def _build_program():
    from contextlib import ExitStack

    import concourse.bass as bass  # noqa: F401
    from concourse import bacc, mybir, tile

    f32 = mybir.dt.float32
    mult = mybir.AluOpType.mult
    add = mybir.AluOpType.add
    Square = mybir.ActivationFunctionType.Square
    Sqrt = mybir.ActivationFunctionType.Sqrt

    nc = bacc.Bacc(
        "TRN2",
        target_bir_lowering=False,
        debug=False,
        enable_asserts=False,
        num_devices=N_CORES,
    )

    a = nc.dram_tensor("a_in", [E, T_CORE, H], f32, kind="ExternalInput").ap()
    tok = nc.dram_tensor("tok_in", [T_CORE, H], f32, kind="ExternalInput").ap()
    res = nc.dram_tensor("res_in", [T_CORE, H], f32, kind="ExternalInput").ap()
    # scales pre-packed host-side as [P, N_CHUNKS*E]: col c*E+e = scale[e, c*128+p]
    sc = nc.dram_tensor("sc_in", [P, N_CHUNKS * E], f32, kind="ExternalInput").ap()
    nw = nc.dram_tensor("nw_in", [P, H], f32, kind="ExternalInput").ap()
    hid_out = nc.dram_tensor("hid_out", [T_CORE, H], f32, kind="ExternalOutput").ap()
    ores_out = nc.dram_tensor("ores_out", [T_CORE, H], f32, kind="ExternalOutput").ap()

    with tile.TileContext(nc) as tc, ExitStack() as ctx:
        nw_pool = ctx.enter_context(tc.tile_pool(name="nw", bufs=1))
        a_pool = ctx.enter_context(tc.tile_pool(name="a", bufs=4))
        tr_pool = ctx.enter_context(tc.tile_pool(name="tr", bufs=3))
        acc_pool = ctx.enter_context(tc.tile_pool(name="acc", bufs=2))
        hid_pool = ctx.enter_context(tc.tile_pool(name="hid", bufs=1))
        st_pool = ctx.enter_context(tc.tile_pool(name="st", bufs=2))
        ps_pool = ctx.enter_context(tc.tile_pool(name="ps", bufs=1, space="PSUM"))

        # one-time preloads on the SWDGE path (keep the HWDGE load FIFO clean)
        sc_t = nw_pool.tile([P, N_CHUNKS * E], f32, tag="sc")
        nc.gpsimd.dma_start(out=sc_t[:], in_=sc[:, :])
        nw_t = nw_pool.tile([P, H], f32)
        nc.gpsimd.dma_start(out=nw_t[:], in_=nw[:, :])

        # const per-partition scalars for ACT bias operands (no const-AP db here)
        zero_t = nw_pool.tile([P, 1], f32, tag="zero")
        nc.vector.memset(zero_t[:], 0.0)
        eps_t = nw_pool.tile([P, 1], f32, tag="eps")
        nc.vector.memset(eps_t[:], EPS)

        for c in range(N_CHUNKS):
            t0 = c * P
            tok_t = tr_pool.tile([P, H], f32, tag="tr")
            nc.sync.dma_start(out=tok_t[:], in_=tok[t0 : t0 + P, :])

            # last chunk runs in two H-halves to shorten the kernel tail
            splits = [(0, H)] if c < N_CHUNKS - 1 else [(0, H // 2), (H // 2, H // 2)]
            acc_t = acc_pool.tile([P, H], f32)
            res_t = None
            var_parts = []
            for off, w in splits:
                cols = slice(off, off + w)
                for e in range(E):
                    a_t = a_pool.tile([P, H], f32, tag="a_t")
                    nc.sync.dma_start(
                        out=a_t[:, 0:w], in_=a[e, t0 : t0 + P, cols]
                    )
                    prev_ap = tok_t[:, cols] if e == 0 else acc_t[:, cols]
                    nc.vector.scalar_tensor_tensor(
                        out=acc_t[:, cols],
                        in0=a_t[:, 0:w],
                        scalar=sc_t[:, c * E + e : c * E + e + 1],
                        in1=prev_ap,
                        op0=mult,
                        op1=add,
                    )
                if res_t is None:
                    # issued after the expert loads: its pool slot frees late in
                    # the previous chunk, and an earlier issue would head-of-line
                    # block the load FIFO on that slot
                    res_t = tr_pool.tile([P, H], f32, tag="tr")
                    nc.sync.dma_start(out=res_t[:], in_=res[t0 : t0 + P, :])
                # residual add on the otherwise-idle GpSimd engine (frees DVE)
                nc.gpsimd.tensor_tensor(
                    out=acc_t[:, cols], in0=acc_t[:, cols], in1=res_t[:, cols], op=add
                )
                nc.gpsimd.dma_start(out=ores_out[t0 : t0 + P, cols], in_=acc_t[:, cols])

                # partial mean-square on ACT: sum(Square(acc/64)) = sum(acc^2)/4096
                var_t = st_pool.tile([P, 1], f32, tag="var")
                sq_t = ps_pool.tile([P, H], f32, tag="sq")
                nc.scalar.activation(
                    out=sq_t[:, 0:w], in_=acc_t[:, cols], func=Square,
                    scale=1.0 / 64.0, bias=zero_t[:, 0:1], accum_out=var_t[:],
                )
                var_parts.append(var_t)

            if len(var_parts) > 1:
                vsum_t = st_pool.tile([P, 1], f32, tag="vsum")
                nc.vector.tensor_tensor(
                    out=vsum_t[:], in0=var_parts[0][:], in1=var_parts[1][:], op=add
                )
                var_t = vsum_t
            else:
                var_t = var_parts[0]

            # rsqrt(var + eps): ACT Sqrt seed + DVE reciprocal + 1 Newton step
            std_t = st_pool.tile([P, 1], f32)
            nc.scalar.activation(
                out=std_t[:], in_=var_t[:], func=Sqrt, bias=eps_t[:, 0:1]
            )
            y_t = st_pool.tile([P, 1], f32)
            nc.vector.reciprocal(out=y_t[:], in_=std_t[:])
            x_t = st_pool.tile([P, 1], f32)
            nc.vector.tensor_scalar_add(x_t[:], var_t[:], EPS)
            t_t = st_pool.tile([P, 1], f32)
            nc.vector.tensor_tensor(out=t_t[:], in0=y_t[:], in1=y_t[:], op=mult)
            nc.vector.tensor_tensor(out=t_t[:], in0=t_t[:], in1=x_t[:], op=mult)
            h_t = st_pool.tile([P, 1], f32)
            nc.vector.tensor_scalar(
                out=h_t[:], in0=t_t[:], scalar1=-0.5, scalar2=1.5, op0=mult, op1=add
            )
            y2_t = st_pool.tile([P, 1], f32)
            nc.vector.tensor_tensor(out=y2_t[:], in0=y_t[:], in1=h_t[:], op=mult)

            hid_t = hid_pool.tile([P, H], f32)
            for off, w in splits:
                cols = slice(off, off + w)
                nc.vector.scalar_tensor_tensor(
                    out=hid_t[:, cols],
                    in0=acc_t[:, cols],
                    scalar=y2_t[:, 0:1],
                    in1=nw_t[:, cols],
                    op0=mult,
                    op1=mult,
                )
                nc.gpsimd.dma_start(out=hid_out[t0 : t0 + P, cols], in_=hid_t[:, cols])

    nc.compile()
    return nc


def _get_program():
    if "nc" not in _CACHE:
        _CACHE["nc"] = _build_program()
    return _CACHE["nc"]


def _make_in_maps(residual, norm_weight, scale_input, active, token_input):
    nw_b = np.ascontiguousarray(
        np.broadcast_to(np.asarray(norm_weight, np.float32), (P, H))
    )
    in_maps = []
    for c in range(N_CORES):
        lo, hi = c * T_CORE, (c + 1) * T_CORE
        in_maps.append(
            {
                "a_in": np.ascontiguousarray(active[:, lo:hi, :], np.float32),
                "tok_in": np.ascontiguousarray(token_input[lo:hi], np.float32),
                "res_in": np.ascontiguousarray(residual[lo:hi], np.float32),
                "sc_in": np.ascontiguousarray(
                    scale_input[:, lo:hi]
                    .reshape(E, N_CHUNKS, P)
                    .transpose(2, 1, 0)
                    .reshape(P, N_CHUNKS * E),
                    np.float32,
                ),
                "nw_in": nw_b,
            }
        )
    return in_maps


def _ensure_ntff_hook():
    """Register the axon NTFF profiling hook if the image's antenv lacks it."""
    import types

    name = "antenv.axon_hooks"
    if name in sys.modules:
        return
    try:
        import antenv.axon_hooks  # noqa: F401

        return
    except ImportError:
        pass
    mod = types.ModuleType(name)
    mod._hook = None
    mod.set_axon_ntff_profile_hook = lambda h: setattr(mod, "_hook", h)
    mod.get_axon_ntff_profile_hook = lambda: mod._hook
    sys.modules[name] = mod
    try:
        from trn_agent_boot.trn_boot import _ntff_profile_via_ctypes

        h = _ntff_profile_via_ctypes("/opt/axon/libaxon_pjrt.so")
        if h is not None:
            mod._hook = h
    except Exception:
        pass


def kernel(
    residual,
    norm_weight,
    scale_input,
    active_experts_token_input,
    token_input,
    device_num_experts,
    _trace=False,
):
    if _trace:
        _ensure_ntff_hook()
    from concourse.bass_utils import run_bass_kernel_spmd

    assert int(device_num_experts) == E
    residual = np.asarray(residual, np.float32)
    norm_weight = np.asarray(norm_weight, np.float32)
    scale_input = np.asarray(scale_input, np.float32)
    active = np.asarray(active_experts_token_input, np.float32)
    token_input = np.asarray(token_input, np.float32)

    nc = _get_program()
    in_maps = _make_in_maps(residual, norm_weight, scale_input, active, token_input)
    r = run_bass_kernel_spmd(nc, in_maps, list(range(N_CORES)), trace=_trace)
    hidden = np.concatenate([r.results[c]["hid_out"] for c in range(N_CORES)], axis=0)
    outres = np.concatenate([r.results[c]["ores_out"] for c in range(N_CORES)], axis=0)
    if _trace:
        _CACHE["last_result"] = r
    return hidden, outres


# revision 18
# speedup vs baseline: 1.2223x; 1.0049x over previous
"""MoE all-reduce + RMSNorm fused kernel for Trainium2 (8 NeuronCores).

Computes, for E=8, T=8192, H=4096 (all fp32):
    expert_reduction = einsum("eth,et->th", active_experts_token_input, scale_input)
    output_residual  = expert_reduction + token_input + residual
    hidden_states    = output_residual * rsqrt(mean(output_residual^2, -1) + 1e-5) * norm_weight
returns (hidden_states, output_residual).

Sharding: tokens (T) split evenly across the 8 cores (data/sequence parallel);
the norm is over H so every core is fully independent — no collectives.

Per-core device program: 8 chunks of 128 tokens (tokens on partitions, H on the
free axis). Per chunk the expert reduction runs as 8 fused DVE
scalar_tensor_tensor MACs (acc = a_e * s_e + acc), the mean-square runs on the
otherwise-idle ACT engine (Square activation with accum_out, scratch in PSUM),
and rsqrt = ACT Sqrt + DVE reciprocal + one Newton step (ACT Sqrt alone is
low-precision). The kernel is HBM-bandwidth-bound: ~192 MiB of DMA per core.
"""

import sys
import numpy as np

try:
    import concourse  # noqa: F401
except ImportError:
    sys.path.insert(0, "/opt/trn_rl_repo")

E, T, H = 8, 8192, 4096
N_CORES = 8
T_CORE = T // N_CORES  # 1024 tokens per core
P = 128                # SBUF partitions = tokens per chunk
N_CHUNKS = T_CORE // P  # 8
EPS = 1e-5

_CACHE = {}


def _build_program():
    from contextlib import ExitStack

    import concourse.bass as bass  # noqa: F401
    from concourse import bacc, mybir, tile

    f32 = mybir.dt.float32
    mult = mybir.AluOpType.mult
    add = mybir.AluOpType.add
    Square = mybir.ActivationFunctionType.Square
    Sqrt = mybir.ActivationFunctionType.Sqrt

    nc = bacc.Bacc(
        "TRN2",
        target_bir_lowering=False,
        debug=False,
        enable_asserts=False,
        num_devices=N_CORES,
    )

    a = nc.dram_tensor("a_in", [E, T_CORE, H], f32, kind="ExternalInput").ap()
    tok = nc.dram_tensor("tok_in", [T_CORE, H], f32, kind="ExternalInput").ap()
    res = nc.dram_tensor("res_in", [T_CORE, H], f32, kind="ExternalInput").ap()
    # scales pre-packed host-side as [P, N_CHUNKS*E]: col c*E+e = scale[e, c*128+p]
    sc = nc.dram_tensor("sc_in", [P, N_CHUNKS * E], f32, kind="ExternalInput").ap()
    nw = nc.dram_tensor("nw_in", [P, H], f32, kind="ExternalInput").ap()
    hid_out = nc.dram_tensor("hid_out", [T_CORE, H], f32, kind="ExternalOutput").ap()
    ores_out = nc.dram_tensor("ores_out", [T_CORE, H], f32, kind="ExternalOutput").ap()

    with tile.TileContext(nc) as tc, ExitStack() as ctx:
        nw_pool = ctx.enter_context(tc.tile_pool(name="nw", bufs=1))
        a_pool = ctx.enter_context(tc.tile_pool(name="a", bufs=4))
        tr_pool = ctx.enter_context(tc.tile_pool(name="tr", bufs=3))
        acc_pool = ctx.enter_context(tc.tile_pool(name="acc", bufs=2))
        hid_pool = ctx.enter_context(tc.tile_pool(name="hid", bufs=1))
        st_pool = ctx.enter_context(tc.tile_pool(name="st", bufs=2))
        ps_pool = ctx.enter_context(tc.tile_pool(name="ps", bufs=1, space="PSUM"))

        # one-time preloads on the SWDGE path (keep the HWDGE load FIFO clean)
        sc_t = nw_pool.tile([P, N_CHUNKS * E], f32, tag="sc")
        nc.gpsimd.dma_start(out=sc_t[:], in_=sc[:, :])
        nw_t = nw_pool.tile([P, H], f32)
        nc.gpsimd.dma_start(out=nw_t[:], in_=nw[:, :])

        # const per-partition scalars for ACT bias operands (no const-AP db here)
        zero_t = nw_pool.tile([P, 1], f32, tag="zero")
        nc.vector.memset(zero_t[:], 0.0)
        eps_t = nw_pool.tile([P, 1], f32, tag="eps")
        nc.vector.memset(eps_t[:], EPS)

        for c in range(N_CHUNKS):
            t0 = c * P
            tok_t = tr_pool.tile([P, H], f32, tag="tr")
            nc.sync.dma_start(out=tok_t[:], in_=tok[t0 : t0 + P, :])

            # last chunk runs in two H-halves to shorten the kernel tail
            splits = [(0, H)] if c < N_CHUNKS - 1 else [(0, H // 2), (H // 2, H // 2)]
            acc_t = acc_pool.tile([P, H], f32)
            res_t = None
            var_parts = []
            for off, w in splits:
                cols = slice(off, off + w)
                for e in range(E):
                    a_t = a_pool.tile([P, H], f32, tag="a_t")
                    nc.sync.dma_start(
                        out=a_t[:, 0:w], in_=a[e, t0 : t0 + P, cols]
                    )
                    prev_ap = tok_t[:, cols] if e == 0 else acc_t[:, cols]
                    nc.vector.scalar_tensor_tensor(
                        out=acc_t[:, cols],
                        in0=a_t[:, 0:w],
                        scalar=sc_t[:, c * E + e : c * E + e + 1],
                        in1=prev_ap,
                        op0=mult,
                        op1=add,
                    )
                if res_t is None:
                    # issued after the expert loads: its pool slot frees late in
                    # the previous chunk, and an earlier issue would head-of-line
                    # block the load FIFO on that slot
                    res_t = tr_pool.tile([P, H], f32, tag="tr")
                    nc.sync.dma_start(out=res_t[:], in_=res[t0 : t0 + P, :])
                nc.vector.tensor_tensor(
                    out=acc_t[:, cols], in0=acc_t[:, cols], in1=res_t[:, cols], op=add
                )
                nc.gpsimd.dma_start(out=ores_out[t0 : t0 + P, cols], in_=acc_t[:, cols])

                # partial mean-square on ACT: sum(Square(acc/64)) = sum(acc^2)/4096
                var_t = st_pool.tile([P, 1], f32, tag="var")
                sq_t = ps_pool.tile([P, H], f32, tag="sq")
                nc.scalar.activation(
                    out=sq_t[:, 0:w], in_=acc_t[:, cols], func=Square,
                    scale=1.0 / 64.0, bias=zero_t[:, 0:1], accum_out=var_t[:],
                )
                var_parts.append(var_t)

            if len(var_parts) > 1:
                vsum_t = st_pool.tile([P, 1], f32, tag="vsum")
                nc.vector.tensor_tensor(
                    out=vsum_t[:], in0=var_parts[0][:], in1=var_parts[1][:], op=add
                )
                var_t = vsum_t
            else:
                var_t = var_parts[0]

            # rsqrt(var + eps): ACT Sqrt seed + DVE reciprocal + 1 Newton step
            std_t = st_pool.tile([P, 1], f32)
            nc.scalar.activation(
                out=std_t[:], in_=var_t[:], func=Sqrt, bias=eps_t[:, 0:1]
            )
            y_t = st_pool.tile([P, 1], f32)
            nc.vector.reciprocal(out=y_t[:], in_=std_t[:])
            x_t = st_pool.tile([P, 1], f32)
            nc.vector.tensor_scalar_add(x_t[:], var_t[:], EPS)
            t_t = st_pool.tile([P, 1], f32)
            nc.vector.tensor_tensor(out=t_t[:], in0=y_t[:], in1=y_t[:], op=mult)
            nc.vector.tensor_tensor(out=t_t[:], in0=t_t[:], in1=x_t[:], op=mult)
            h_t = st_pool.tile([P, 1], f32)
            nc.vector.tensor_scalar(
                out=h_t[:], in0=t_t[:], scalar1=-0.5, scalar2=1.5, op0=mult, op1=add
            )
            y2_t = st_pool.tile([P, 1], f32)
            nc.vector.tensor_tensor(out=y2_t[:], in0=y_t[:], in1=h_t[:], op=mult)

            hid_t = hid_pool.tile([P, H], f32)
            for off, w in splits:
                cols = slice(off, off + w)
                nc.vector.scalar_tensor_tensor(
                    out=hid_t[:, cols],
                    in0=acc_t[:, cols],
                    scalar=y2_t[:, 0:1],
                    in1=nw_t[:, cols],
                    op0=mult,
                    op1=mult,
                )
                nc.gpsimd.dma_start(out=hid_out[t0 : t0 + P, cols], in_=hid_t[:, cols])

    nc.compile()
    return nc


def _get_program():
    if "nc" not in _CACHE:
        _CACHE["nc"] = _build_program()
    return _CACHE["nc"]


def _make_in_maps(residual, norm_weight, scale_input, active, token_input):
    nw_b = np.ascontiguousarray(
        np.broadcast_to(np.asarray(norm_weight, np.float32), (P, H))
    )
    in_maps = []
    for c in range(N_CORES):
        lo, hi = c * T_CORE, (c + 1) * T_CORE
        in_maps.append(
            {
                "a_in": np.ascontiguousarray(active[:, lo:hi, :], np.float32),
                "tok_in": np.ascontiguousarray(token_input[lo:hi], np.float32),
                "res_in": np.ascontiguousarray(residual[lo:hi], np.float32),
                "sc_in": np.ascontiguousarray(
                    scale_input[:, lo:hi]
                    .reshape(E, N_CHUNKS, P)
                    .transpose(2, 1, 0)
                    .reshape(P, N_CHUNKS * E),
                    np.float32,
                ),
                "nw_in": nw_b,
            }
        )
    return in_maps


def _ensure_ntff_hook():
    """Register the axon NTFF profiling hook if the image's antenv lacks it."""
    import types

    name = "antenv.axon_hooks"
    if name in sys.modules:
        return
    try:
        import antenv.axon_hooks  # noqa: F401

        return
    except ImportError:
        pass
    mod = types.ModuleType(name)
    mod._hook = None
    mod.set_axon_ntff_profile_hook = lambda h: setattr(mod, "_hook", h)
    mod.get_axon_ntff_profile_hook = lambda: mod._hook
    sys.modules[name] = mod
    try:
        from trn_agent_boot.trn_boot import _ntff_profile_via_ctypes

        h = _ntff_profile_via_ctypes("/opt/axon/libaxon_pjrt.so")
        if h is not None:
            mod._hook = h
    except Exception:
        pass


def kernel(
    residual,
    norm_weight,
    scale_input,
    active_experts_token_input,
    token_input,
    device_num_experts,
    _trace=False,
):
    if _trace:
        _ensure_ntff_hook()
    from concourse.bass_utils import run_bass_kernel_spmd

    assert int(device_num_experts) == E
    residual = np.asarray(residual, np.float32)
    norm_weight = np.asarray(norm_weight, np.float32)
    scale_input = np.asarray(scale_input, np.float32)
    active = np.asarray(active_experts_token_input, np.float32)
    token_input = np.asarray(token_input, np.float32)

    nc = _get_program()
    in_maps = _make_in_maps(residual, norm_weight, scale_input, active, token_input)
    r = run_bass_kernel_spmd(nc, in_maps, list(range(N_CORES)), trace=_trace)
    hidden = np.concatenate([r.results[c]["hid_out"] for c in range(N_CORES)], axis=0)
    outres = np.concatenate([r.results[c]["ores_out"] for c in range(N_CORES)], axis=0)
    if _trace:
        _CACHE["last_result"] = r
    return hidden, outres
